# revision 7
# baseline (speedup 1.0000x reference)
"""MoE top-2 routing kernel for Trainium2 (8 NeuronCores).

v4 (mixed precision: rank-1 bf16, rank-2 fp8 E4M3 DoubleRow; W-stationary,
balanced expert-parallel) with finer load balancing: each core carries THREE
token segments per precision class (two A-slots of cap sa + one B-slot of
cap sb; 16 A + 8 B slots per class fleet-wide). That drops per-core capacity
from (1104 bf16 + 1056 fp8) to ~(1040 + 1040), i.e. ~2% over the perfect
2x1024 mean. PSUM chunk names are shared between the two phases (they are
temporally disjoint), so bank pressure stays low.

See kernel_v4.py docstring for the precision/error analysis (measured
end-to-end rel err ~1.5e-2 vs the 2e-2 gate; bf16-only is ~2.0e-3).
"""

import os

import numpy as np

N_TOK, N_EXP, D_IN, D_OUT = 8192, 8, 2048, 2048
TOP_K = 2

P = 128
KSUB = D_IN // P       # 16 contraction subtiles
KPAIR = KSUB // 2      # 8 DoubleRow contraction pairs
OB = D_OUT // P        # 16 out-feature blocks
MAXMOV = 512           # max moving free dim per matmul
WPRE = 4               # out-blocks of W slices to prefetch ahead
X8_SCALE = 16.0
W8_SCALE = 1024.0

LAST_EXEC_NS = None  # set when KERNEL_TRACE=1

_cache = {}


def _install_ntff_shim():
    """Provide antenv.axon_hooks (missing in this image) so trace=True works."""
    import sys
    import types

    if "antenv.axon_hooks" in sys.modules:
        return
    try:
        import antenv
        from trn_agent_boot.trn_boot import _ntff_profile_via_ctypes

        mod = types.ModuleType("antenv.axon_hooks")
        mod._hook = _ntff_profile_via_ctypes("/opt/axon/libaxon_pjrt.so")
        mod.set_axon_ntff_profile_hook = lambda h: setattr(mod, "_hook", h)
        mod.get_axon_ntff_profile_hook = lambda: mod._hook
        sys.modules["antenv.axon_hooks"] = mod
        antenv.axon_hooks = mod
    except Exception:
        pass


def _chunks(t0, seg_len):
    """Split [t0, t0+seg_len) into equal-ish 16-aligned chunks <= MAXMOV."""
    if seg_len <= 0:
        return []
    n = -(-seg_len // MAXMOV)
    base = -(--(-seg_len // n) // 16) * 16
    out = []
    t = 0
    while t < seg_len:
        ln = min(base, seg_len - t)
        out.append((t0 + t, ln))
        t += ln
    return out


def _feasible(counts, na, sa, nb, sb):
    """Cover counts with na A-slots (cap sa) + nb B-slots (cap sb)?"""
    states = {(0, 0): None}
    choices = []
    for cnt in counts:
        nxt = {}
        for (au, bu), _ in states.items():
            for a in range(0, na - au + 1):
                rem = cnt - a * sa
                b = 0 if rem <= 0 else (-(-rem // sb) if sb > 0 else None)
                if b is None or bu + b > nb:
                    continue
                key = (au + a, bu + b)
                if key not in nxt:
                    nxt[key] = ((au, bu), (a, b))
        if not nxt:
            return None
        choices.append(nxt)
        states = nxt
    key = min(states)  # unused slots become zero-length pieces
    out = []
    for lvl in reversed(choices):
        prev, ab = lvl[key]
        out.append(ab)
        key = prev
    out.reverse()
    return out


def _balance(counts, n_cores=8, quant=16):
    """Min-capacity assignment onto per-core slot structures.

    Tries several per-core compositions (1-2 A-slots of cap sa + 1-2 B-slots
    of cap sb, uniform across cores) and keeps the minimum total per-core
    capacity. Returns (caps, cores): caps is the per-core segment cap tuple;
    cores[i] is the matching list of pieces (e, off, ln).
    """
    cmax = max(max(counts), 2)
    rq = lambda v: -(-v // quant) * quant
    best = None  # (C, na_per, nb_per, sa, sb, ab)
    # >2 segments/core means another full W stream per out-block; measured:
    # 3 bf16 W streams exceed per-core DMA bandwidth (18.6us of PE stalls
    # for a 5us capacity gain). Keep one A-slot + one B-slot per core.
    for na_per, nb_per in ((1, 1),):
        na, nb = na_per * n_cores, nb_per * n_cores
        nseg = na_per + nb_per
        for sa in range(rq(-(-cmax // nseg)), rq(cmax) + quant, quant):
            if best is not None and na_per * sa + nb_per * quant >= best[0]:
                break
            lo, hi = quant, sa
            got = None
            while lo <= hi:
                mid = (lo + hi) // 2
                ab = _feasible(counts, na, sa, nb, mid)
                if ab is not None:
                    got = (mid, ab)
                    hi = mid - 1
                else:
                    lo = mid + 1
            if got is not None:
                sb, ab = rq(got[0]), got[1]
                C = na_per * sa + nb_per * sb
                if best is None or C < best[0]:
                    best = (C, na_per, nb_per, sa, sb, ab)
    if best is None:  # fallback: pure expert-parallel halves
        sa = rq(-(-cmax // 2))
        best = (2 * sa, 1, 1, sa, sa, [(1, 1)] * len(counts))
    _, na_per, nb_per, sa, sb, ab = best
    na, nb = na_per * n_cores, nb_per * n_cores

    a_pieces, b_pieces = [], []
    for e, (cnt, (a, b)) in enumerate(zip(counts, ab)):
        off = 0
        need = cnt
        slots = [("A", sa)] * a + [("B", sb)] * b
        caps = sum(s for _, s in slots)
        for kind, cap in slots:
            take = min(cap, need)
            caps -= cap
            if need - take > caps:
                take = need - caps
            (a_pieces if kind == "A" else b_pieces).append((e, off, take))
            off += take
            need -= take
        assert need == 0, (e, cnt, ab)
    while len(a_pieces) < na:
        a_pieces.append((0, counts[0], 0))
    while len(b_pieces) < nb:
        b_pieces.append((0, counts[0], 0))
    caps = (sa,) * na_per + (sb,) * nb_per
    cores = []
    for i in range(n_cores):
        cores.append(a_pieces[na_per * i:na_per * (i + 1)]
                     + b_pieces[nb_per * i:nb_per * (i + 1)])
    return caps, cores


def _build_v5(caps1, caps2):
    """Two-phase kernel: bf16 segments then fp8-DoubleRow segments."""
    import concourse.mybir as mybir
    import concourse.tile as tile
    from concourse import bacc

    C1, C2 = sum(caps1), sum(caps2)
    n1, n2 = len(caps1), len(caps2)
    offs1 = [sum(caps1[:i]) for i in range(n1)]
    offs2 = [sum(caps2[:i]) for i in range(n2)]
    segs1 = [_chunks(offs1[s], caps1[s]) for s in range(n1)]
    segs2 = [_chunks(offs2[s], caps2[s]) for s in range(n2)]
    n_ps = max(sum(len(s) for s in segs1), sum(len(s) for s in segs2))
    assert n_ps <= 7, (caps1, caps2)

    nc = bacc.Bacc("TRN2", target_bir_lowering=False, debug=False)
    with tile.TileContext(nc) as tc:
        with (
            tc.tile_pool(name="dram", bufs=1, space="DRAM") as dram,
            tc.tile_pool(name="wbpool", bufs=n1 * (WPRE + 1)) as wbpool,
            tc.tile_pool(name="w8pool", bufs=n2 * (WPRE + 1)) as w8pool,
            tc.tile_pool(name="xpool", bufs=1) as xpool,
            tc.tile_pool(name="opool", bufs=3) as opool,
            tc.tile_pool(name="pspool", bufs=1, space="PSUM") as pspool,
            tc.tile_pool(name="warmpool", bufs=1) as warmpool,
            tc.tile_pool(name="warmps", bufs=1, space="PSUM") as warmps,
        ):
            xb = dram.tile([P, KSUB, C1], mybir.dt.bfloat16,
                           kind="ExternalInput", name="xb")
            x8 = dram.tile([P, KSUB, C2], mybir.dt.float8e4,
                           kind="ExternalInput", name="x8")
            wbsegs = [dram.tile([OB, P, KSUB, P], mybir.dt.bfloat16,
                                kind="ExternalInput", name=f"wb{s}")
                      for s in range(n1)]
            w8segs = [dram.tile([OB, P, KSUB, P], mybir.dt.float8e4,
                                kind="ExternalInput", name=f"w8{s}")
                      for s in range(n2)]
            out = dram.tile([OB, P, C1 + C2], mybir.dt.bfloat16,
                            kind="ExternalOutput", name="out")

            # PE warmup: dummy matmuls during the initial DMA wait ramp the
            # HAM clock before the first real matmul issues.
            wl = warmpool.tile([P, P], mybir.dt.bfloat16, name="warm_l")
            wr = warmpool.tile([P, MAXMOV], mybir.dt.bfloat16, name="warm_r")
            nc.gpsimd.memset(wl[:], 0.0)
            nc.gpsimd.memset(wr[:], 0.0)
            wp = warmps.tile([P, MAXMOV], mybir.dt.float32, name="warm_p")
            for _ in range(16):
                nc.tensor.matmul(wp[:], lhsT=wl[:], rhs=wr[:], start=True,
                                 stop=True, skip_group_check=True)

            wbt, w8t = {}, {}

            def load_wb(ob):
                for s in range(n1):
                    if not segs1[s]:
                        continue
                    t = wbpool.tile([P, KSUB, P], mybir.dt.bfloat16, name="wbt")
                    nc.sync.dma_start(out=t[:], in_=wbsegs[s][ob])
                    wbt[(ob, s)] = t

            def load_w8(ob):
                for s in range(n2):
                    if not segs2[s]:
                        continue
                    t = w8pool.tile([P, KSUB, P], mybir.dt.float8e4, name="w8t")
                    nc.sync.dma_start(out=t[:], in_=w8segs[s][ob])
                    w8t[(ob, s)] = t

            load_wb(0)
            xbt = xpool.tile([P, KSUB, C1], mybir.dt.bfloat16, name="xbt")
            for ks in range(KSUB):
                nc.sync.dma_start(out=xbt[:, ks], in_=xb[:, ks])
            for ob in range(1, WPRE + 1):
                load_wb(ob)
            x8t = xpool.tile([P, KSUB, C2], mybir.dt.float8e4, name="x8t")

            # ---- phase 1: bf16 (rank-1) ----
            for ob in range(OB):
                if ob + WPRE + 1 < OB:
                    load_wb(ob + WPRE + 1)
                # x8 input rides behind the early bf16 phase, after the
                # startup burst of xb/wb DMAs has drained
                if ob == 2:
                    for ks in range(KSUB):
                        nc.sync.dma_start(out=x8t[:, ks], in_=x8[:, ks])
                if ob < WPRE + 1:  # fp8 W prefetch rides behind phase 1
                    load_w8(ob)
                ot = opool.tile([P, C1], mybir.dt.bfloat16, name="ot1")
                ci = 0
                for s in range(n1):
                    if not segs1[s]:
                        continue
                    wt = wbt.pop((ob, s))
                    for (t0, ln) in segs1[s]:
                        ps = pspool.tile([P, MAXMOV], mybir.dt.float32,
                                         name=f"ps{ci}")
                        ci += 1
                        for k in range(KSUB):
                            nc.tensor.matmul(
                                ps[:, :ln],
                                lhsT=wt[:, k, :],
                                rhs=xbt[:, k, t0:t0 + ln],
                                start=(k == 0),
                                stop=(k == KSUB - 1),
                            )
                        nc.vector.tensor_copy(ot[:, t0:t0 + ln], ps[:, :ln])
                    nc.sync.dma_start(
                        out=out[ob, :, offs1[s]:offs1[s] + caps1[s]],
                        in_=ot[:, offs1[s]:offs1[s] + caps1[s]])

            # ---- phase 2: fp8 DoubleRow (rank-2) ----
            dr = mybir.MatmulPerfMode.DoubleRow
            for ob in range(OB):
                if ob + WPRE + 1 < OB:
                    load_w8(ob + WPRE + 1)
                ot = opool.tile([P, C2], mybir.dt.bfloat16, name="ot2")
                last_ob = ob == OB - 1
                ci = 0
                for s in range(n2):
                    if not segs2[s]:
                        continue
                    wt = w8t.pop((ob, s))
                    for (t0, ln) in segs2[s]:
                        ps = pspool.tile([P, MAXMOV], mybir.dt.float32,
                                         name=f"ps{ci}")
                        ci += 1
                        for kp in range(KPAIR):
                            nc.tensor.matmul(
                                ps[:, :ln],
                                lhsT=wt[:, 2 * kp:2 * kp + 2, :],
                                rhs=x8t[:, 2 * kp:2 * kp + 2, t0:t0 + ln],
                                start=(kp == 0),
                                stop=(kp == KPAIR - 1),
                                perf_mode=dr,
                            )
                        nc.vector.tensor_copy(ot[:, t0:t0 + ln], ps[:, :ln])
                        if last_ob:
                            # per-chunk store: the tail drains during the
                            # remaining chunks' evictions
                            nc.sync.dma_start(
                                out=out[ob, :, C1 + t0:C1 + t0 + ln],
                                in_=ot[:, t0:t0 + ln])
                    if not last_ob:
                        nc.sync.dma_start(
                            out=out[ob, :, C1 + offs2[s]:C1 + offs2[s] + caps2[s]],
                            in_=ot[:, offs2[s]:offs2[s] + caps2[s]])

    nc.compile()
    return (nc, xb.name, x8.name, [w.name for w in wbsegs],
            [w.name for w in w8segs], out.name)


def kernel(X, G, W, b):
    global LAST_EXEC_NS
    from concourse.bass_utils import run_bass_kernel_spmd
    import ml_dtypes

    X = np.ascontiguousarray(np.asarray(X, dtype=np.float32))
    G = np.asarray(G, dtype=np.float32)
    W = np.asarray(W, dtype=np.float32)
    b = np.asarray(b, dtype=np.float32)
    n_tok, d_in = X.shape
    n_exp = G.shape[1]
    d_out = W.shape[1]

    # --- host gating: softmax over experts, top-2, renormalize ---
    g = G - G.max(axis=1, keepdims=True)
    sm = np.exp(g)
    sm /= sm.sum(axis=1, keepdims=True)
    top_idx = np.argsort(-sm, axis=1, kind="stable")[:, :TOP_K]  # ties -> lower index
    top_w = np.take_along_axis(sm, top_idx, axis=1)
    norm_w = top_w / top_w.sum(axis=1, keepdims=True)

    # --- dispatch by (expert, rank): rank-1 -> bf16 class, rank-2 -> fp8 ---
    cls_tokens = [[], []]
    cls_scales = [[], []]
    for e in range(n_exp):
        for k in range(TOP_K):
            m = top_idx[:, k] == e
            cl = 0 if k == 0 else 1
            cls_tokens[cl].append(np.where(m)[0])
            cls_scales[cl].append(norm_w[m, k].astype(np.float32))

    counts1 = [len(i) for i in cls_tokens[0]]
    counts2 = [len(i) for i in cls_tokens[1]]
    caps1, cores1 = _balance(counts1, n_cores=n_exp)
    caps2, cores2 = _balance(counts2, n_cores=n_exp)
    C1, C2 = sum(caps1), sum(caps2)
    offs1 = [sum(caps1[:i]) for i in range(len(caps1))]
    offs2 = [sum(caps2[:i]) for i in range(len(caps2))]

    _install_ntff_shim()  # harmless if unavailable; needed if tracing is on
    key = (caps1, caps2)
    if key not in _cache:
        _cache[key] = _build_v5(caps1, caps2)
    nc, xb_name, x8_name, wb_names, w8_names, out_name = _cache[key]

    # W tiled per expert, cached across cores:
    # w_t[ob, p, ks, o2] = W_e[ob*128+o2, ks*128+p]
    wb_tiled, w8_tiled = {}, {}

    def tile_w(e):
        return np.ascontiguousarray(
            W[e].reshape(OB, P, KSUB, P).transpose(0, 3, 2, 1))

    def get_wb(e):
        if e not in wb_tiled:
            wb_tiled[e] = tile_w(e).astype(ml_dtypes.bfloat16)
        return wb_tiled[e]

    def get_w8(e):
        if e not in w8_tiled:
            w8_tiled[e] = (tile_w(e) * W8_SCALE).astype(ml_dtypes.float8_e4m3)
        return w8_tiled[e]

    def build_x(pieces, offs, Csz, toks, scale):
        Xg = np.zeros((Csz, d_in), dtype=np.float32)
        for (e, off, ln), so in zip(pieces, offs):
            Xg[so:so + ln] = X[toks[e][off:off + ln]]
        if scale != 1.0:
            Xg *= scale
        return np.ascontiguousarray(Xg.reshape(Csz, KSUB, P).transpose(2, 1, 0))

    in_maps = []
    for ci in range(n_exp):
        xb_t = build_x(cores1[ci], offs1, C1, cls_tokens[0], 1.0).astype(
            ml_dtypes.bfloat16)
        x8_t = build_x(cores2[ci], offs2, C2, cls_tokens[1], X8_SCALE).astype(
            ml_dtypes.float8_e4m3)
        m = {xb_name: xb_t, x8_name: x8_t}
        for s in range(len(caps1)):
            m[wb_names[s]] = get_wb(cores1[ci][s][0])
        for s in range(len(caps2)):
            m[w8_names[s]] = get_w8(cores2[ci][s][0])
        in_maps.append(m)

    trace = bool(os.environ.get("KERNEL_TRACE"))
    res = run_bass_kernel_spmd(nc, in_maps, core_ids=list(range(n_exp)),
                               trace=trace)
    LAST_EXEC_NS = res.exec_time_ns

    # --- host combine: scatter-add with gate scale and bias ---
    inv8 = 1.0 / (X8_SCALE * W8_SCALE)
    Y = np.zeros((n_tok, d_out), dtype=np.float32)
    for ci in range(n_exp):
        arr = res.results[ci][out_name].reshape(d_out, C1 + C2).astype(
            np.float32)
        for cl, cores, offs, r_base, sc in (
            (0, cores1, offs1, 0, 1.0),
            (1, cores2, offs2, C1, inv8),
        ):
            for (e, off, ln), so in zip(cores[ci], offs):
                if ln == 0:
                    continue
                idx = cls_tokens[cl][e][off:off + ln]
                s = cls_scales[cl][e][off:off + ln][:, None]
                Ye = arr[:, r_base + so:r_base + so + ln].T
                if sc != 1.0:
                    Ye = Ye * sc
                Y[idx] += s * (Ye + b[e][None, :])
    return Y


# revision 8
# speedup vs baseline: 1.0204x; 1.0204x over previous
"""MoE top-2 routing kernel for Trainium2 (8 NeuronCores).

Mixed-precision balanced expert-parallel: host does gating (softmax + top-2
+ renorm) and token dispatch; rank-1 expert contributions (gate weight
w1 >= 0.5) run in bf16, rank-2 contributions in fp8 E4M3 with DoubleRow
perf mode (2 fp8 weights per PE cell -> 2x streaming rate). The fp8
quantization error lands only on the low-gate-weight term: measured
end-to-end rel err ~1.55e-2 vs the 2e-2 gate (bf16-only is ~2.0e-3).

Each core carries two token segments per precision class (caps uniform
across cores so the SPMD program is identical; a min-capacity DP splits
heavy experts across cores: 1104 bf16 + 1056 fp8 tokens/core vs 2304 for
naive max-expert padding). More segments would balance better but each
extra segment adds a full W stream per out-block and phase-1 already runs
near per-core DMA bandwidth (measured: 3 bf16 W streams -> 18.6us of PE
stalls for a 5us capacity gain).

Device GEMM is W-stationary: stationary = W[128k x 128out] slices streamed
from DRAM (each used once), moving = SBUF-resident X token chunks (<=512).
k-chains run back-to-back into one PSUM bank per chunk (interleaving banks
between accumulation groups costs ~35ns/matmul in PE micro-idles); weight
loads hide behind the PE background weight buffer (bf16/fp8 enable FWL and
ldweights pull-ahead; fp32r cannot and eats ~54us). PE warmup matmuls must
be bf16, not fp32 (fp32 is quarter-rate: ~853ns each). Output returns as
bf16 and is upcast/combined (gate scale + bias) on host in fp32.
"""

import os

import numpy as np

N_TOK, N_EXP, D_IN, D_OUT = 8192, 8, 2048, 2048
TOP_K = 2

P = 128
KSUB = D_IN // P       # 16 contraction subtiles
KPAIR = KSUB // 2      # 8 DoubleRow contraction pairs
OB = D_OUT // P        # 16 out-feature blocks
MAXMOV = 512           # max moving free dim per matmul
WPRE = 4               # out-blocks of W slices to prefetch ahead
X8_SCALE = 16.0
W8_SCALE = 1024.0

LAST_EXEC_NS = None  # set when KERNEL_TRACE=1

_cache = {}


def _install_ntff_shim():
    """Provide antenv.axon_hooks (missing in this image) so trace=True works."""
    import sys
    import types

    if "antenv.axon_hooks" in sys.modules:
        return
    try:
        import antenv
        from trn_agent_boot.trn_boot import _ntff_profile_via_ctypes

        mod = types.ModuleType("antenv.axon_hooks")
        mod._hook = _ntff_profile_via_ctypes("/opt/axon/libaxon_pjrt.so")
        mod.set_axon_ntff_profile_hook = lambda h: setattr(mod, "_hook", h)
        mod.get_axon_ntff_profile_hook = lambda: mod._hook
        sys.modules["antenv.axon_hooks"] = mod
        antenv.axon_hooks = mod
    except Exception:
        pass


def _chunks(t0, seg_len):
    """Split [t0, t0+seg_len) into equal-ish 16-aligned chunks <= MAXMOV."""
    if seg_len <= 0:
        return []
    n = -(-seg_len // MAXMOV)
    base = -(--(-seg_len // n) // 16) * 16
    out = []
    t = 0
    while t < seg_len:
        ln = min(base, seg_len - t)
        out.append((t0 + t, ln))
        t += ln
    return out


def _feasible(counts, na, sa, nb, sb):
    """Cover counts with na A-slots (cap sa) + nb B-slots (cap sb)?"""
    states = {(0, 0): None}
    choices = []
    for cnt in counts:
        nxt = {}
        for (au, bu), _ in states.items():
            for a in range(0, na - au + 1):
                rem = cnt - a * sa
                b = 0 if rem <= 0 else (-(-rem // sb) if sb > 0 else None)
                if b is None or bu + b > nb:
                    continue
                key = (au + a, bu + b)
                if key not in nxt:
                    nxt[key] = ((au, bu), (a, b))
        if not nxt:
            return None
        choices.append(nxt)
        states = nxt
    key = min(states)  # unused slots become zero-length pieces
    out = []
    for lvl in reversed(choices):
        prev, ab = lvl[key]
        out.append(ab)
        key = prev
    out.reverse()
    return out


def _balance(counts, n_cores=8, quant=16):
    """Min-capacity assignment onto per-core slot structures.

    Tries several per-core compositions (1-2 A-slots of cap sa + 1-2 B-slots
    of cap sb, uniform across cores) and keeps the minimum total per-core
    capacity. Returns (caps, cores): caps is the per-core segment cap tuple;
    cores[i] is the matching list of pieces (e, off, ln).
    """
    cmax = max(max(counts), 2)
    rq = lambda v: -(-v // quant) * quant
    best = None  # (C, na_per, nb_per, sa, sb, ab)
    # >2 segments/core means another full W stream per out-block; measured:
    # 3 bf16 W streams exceed per-core DMA bandwidth (18.6us of PE stalls
    # for a 5us capacity gain). Keep one A-slot + one B-slot per core.
    for na_per, nb_per in ((1, 1),):
        na, nb = na_per * n_cores, nb_per * n_cores
        nseg = na_per + nb_per
        for sa in range(rq(-(-cmax // nseg)), rq(cmax) + quant, quant):
            if best is not None and na_per * sa + nb_per * quant >= best[0]:
                break
            lo, hi = quant, sa
            got = None
            while lo <= hi:
                mid = (lo + hi) // 2
                ab = _feasible(counts, na, sa, nb, mid)
                if ab is not None:
                    got = (mid, ab)
                    hi = mid - 1
                else:
                    lo = mid + 1
            if got is not None:
                sb, ab = rq(got[0]), got[1]
                C = na_per * sa + nb_per * sb
                if best is None or C < best[0]:
                    best = (C, na_per, nb_per, sa, sb, ab)
    if best is None:  # fallback: pure expert-parallel halves
        sa = rq(-(-cmax // 2))
        best = (2 * sa, 1, 1, sa, sa, [(1, 1)] * len(counts))
    _, na_per, nb_per, sa, sb, ab = best
    na, nb = na_per * n_cores, nb_per * n_cores

    a_pieces, b_pieces = [], []
    for e, (cnt, (a, b)) in enumerate(zip(counts, ab)):
        off = 0
        need = cnt
        slots = [("A", sa)] * a + [("B", sb)] * b
        caps = sum(s for _, s in slots)
        for kind, cap in slots:
            take = min(cap, need)
            caps -= cap
            if need - take > caps:
                take = need - caps
            (a_pieces if kind == "A" else b_pieces).append((e, off, take))
            off += take
            need -= take
        assert need == 0, (e, cnt, ab)
    while len(a_pieces) < na:
        a_pieces.append((0, counts[0], 0))
    while len(b_pieces) < nb:
        b_pieces.append((0, counts[0], 0))
    caps = (sa,) * na_per + (sb,) * nb_per
    cores = []
    for i in range(n_cores):
        cores.append(a_pieces[na_per * i:na_per * (i + 1)]
                     + b_pieces[nb_per * i:nb_per * (i + 1)])
    return caps, cores


def _build_v5(caps1, caps2):
    """Two-phase kernel: bf16 segments then fp8-DoubleRow segments."""
    import concourse.mybir as mybir
    import concourse.tile as tile
    from concourse import bacc

    C1, C2 = sum(caps1), sum(caps2)
    n1, n2 = len(caps1), len(caps2)
    offs1 = [sum(caps1[:i]) for i in range(n1)]
    offs2 = [sum(caps2[:i]) for i in range(n2)]
    segs1 = [_chunks(offs1[s], caps1[s]) for s in range(n1)]
    segs2 = [_chunks(offs2[s], caps2[s]) for s in range(n2)]
    n_ps = max(sum(len(s) for s in segs1), sum(len(s) for s in segs2))
    assert n_ps <= 7, (caps1, caps2)

    nc = bacc.Bacc("TRN2", target_bir_lowering=False, debug=False)
    with tile.TileContext(nc) as tc:
        with (
            tc.tile_pool(name="dram", bufs=1, space="DRAM") as dram,
            tc.tile_pool(name="wbpool", bufs=n1 * (WPRE + 1)) as wbpool,
            tc.tile_pool(name="w8pool", bufs=n2 * (WPRE + 1)) as w8pool,
            tc.tile_pool(name="xpool", bufs=1) as xpool,
            tc.tile_pool(name="opool", bufs=3) as opool,
            tc.tile_pool(name="pspool", bufs=1, space="PSUM") as pspool,
            tc.tile_pool(name="warmpool", bufs=1) as warmpool,
            tc.tile_pool(name="warmps", bufs=1, space="PSUM") as warmps,
        ):
            xb = dram.tile([P, KSUB, C1], mybir.dt.bfloat16,
                           kind="ExternalInput", name="xb")
            x8 = dram.tile([P, KSUB, C2], mybir.dt.float8e4,
                           kind="ExternalInput", name="x8")
            wbsegs = [dram.tile([OB, P, KSUB, P], mybir.dt.bfloat16,
                                kind="ExternalInput", name=f"wb{s}")
                      for s in range(n1)]
            w8segs = [dram.tile([OB, P, KSUB, P], mybir.dt.float8e4,
                                kind="ExternalInput", name=f"w8{s}")
                      for s in range(n2)]
            out = dram.tile([OB, P, C1 + C2], mybir.dt.bfloat16,
                            kind="ExternalOutput", name="out")

            # PE warmup: dummy matmuls during the initial DMA wait ramp the
            # HAM clock before the first real matmul issues.
            wl = warmpool.tile([P, P], mybir.dt.bfloat16, name="warm_l")
            wr = warmpool.tile([P, MAXMOV], mybir.dt.bfloat16, name="warm_r")
            nc.gpsimd.memset(wl[:], 0.0)
            nc.gpsimd.memset(wr[:], 0.0)
            wp = warmps.tile([P, MAXMOV], mybir.dt.float32, name="warm_p")
            for _ in range(16):
                nc.tensor.matmul(wp[:], lhsT=wl[:], rhs=wr[:], start=True,
                                 stop=True, skip_group_check=True)

            wbt, w8t = {}, {}

            def load_wb(ob):
                for s in range(n1):
                    if not segs1[s]:
                        continue
                    t = wbpool.tile([P, KSUB, P], mybir.dt.bfloat16, name="wbt")
                    nc.sync.dma_start(out=t[:], in_=wbsegs[s][ob])
                    wbt[(ob, s)] = t

            def load_w8(ob):
                for s in range(n2):
                    if not segs2[s]:
                        continue
                    t = w8pool.tile([P, KSUB, P], mybir.dt.float8e4, name="w8t")
                    nc.sync.dma_start(out=t[:], in_=w8segs[s][ob])
                    w8t[(ob, s)] = t

            load_wb(0)
            xbt = xpool.tile([P, KSUB, C1], mybir.dt.bfloat16, name="xbt")
            for ks in range(KSUB):
                nc.sync.dma_start(out=xbt[:, ks], in_=xb[:, ks])
            for ob in range(1, WPRE + 1):
                load_wb(ob)
            x8t = xpool.tile([P, KSUB, C2], mybir.dt.float8e4, name="x8t")

            # ---- phase 1: bf16 (rank-1) ----
            for ob in range(OB):
                if ob + WPRE + 1 < OB:
                    load_wb(ob + WPRE + 1)
                # x8 input rides behind the early bf16 phase, after the
                # startup burst of xb/wb DMAs has drained
                if ob == 2:
                    for ks in range(KSUB):
                        nc.sync.dma_start(out=x8t[:, ks], in_=x8[:, ks])
                if ob < WPRE + 1:  # fp8 W prefetch rides behind phase 1
                    load_w8(ob)
                ot = opool.tile([P, C1], mybir.dt.bfloat16, name="ot1")
                ci = 0
                for s in range(n1):
                    if not segs1[s]:
                        continue
                    wt = wbt.pop((ob, s))
                    for (t0, ln) in segs1[s]:
                        ps = pspool.tile([P, MAXMOV], mybir.dt.float32,
                                         name=f"ps{ci}")
                        ci += 1
                        for k in range(KSUB):
                            nc.tensor.matmul(
                                ps[:, :ln],
                                lhsT=wt[:, k, :],
                                rhs=xbt[:, k, t0:t0 + ln],
                                start=(k == 0),
                                stop=(k == KSUB - 1),
                            )
                        nc.vector.tensor_copy(ot[:, t0:t0 + ln], ps[:, :ln])
                    nc.sync.dma_start(
                        out=out[ob, :, offs1[s]:offs1[s] + caps1[s]],
                        in_=ot[:, offs1[s]:offs1[s] + caps1[s]])

            # ---- phase 2: fp8 DoubleRow (rank-2) ----
            dr = mybir.MatmulPerfMode.DoubleRow
            for ob in range(OB):
                if ob + WPRE + 1 < OB:
                    load_w8(ob + WPRE + 1)
                ot = opool.tile([P, C2], mybir.dt.bfloat16, name="ot2")
                last_ob = ob == OB - 1
                ci = 0
                for s in range(n2):
                    if not segs2[s]:
                        continue
                    wt = w8t.pop((ob, s))
                    for (t0, ln) in segs2[s]:
                        ps = pspool.tile([P, MAXMOV], mybir.dt.float32,
                                         name=f"ps{ci}")
                        ci += 1
                        for kp in range(KPAIR):
                            nc.tensor.matmul(
                                ps[:, :ln],
                                lhsT=wt[:, 2 * kp:2 * kp + 2, :],
                                rhs=x8t[:, 2 * kp:2 * kp + 2, t0:t0 + ln],
                                start=(kp == 0),
                                stop=(kp == KPAIR - 1),
                                perf_mode=dr,
                            )
                        nc.vector.tensor_copy(ot[:, t0:t0 + ln], ps[:, :ln])
                        if last_ob:
                            # per-chunk store: the tail drains during the
                            # remaining chunks' evictions
                            nc.sync.dma_start(
                                out=out[ob, :, C1 + t0:C1 + t0 + ln],
                                in_=ot[:, t0:t0 + ln])
                    if not last_ob:
                        nc.sync.dma_start(
                            out=out[ob, :, C1 + offs2[s]:C1 + offs2[s] + caps2[s]],
                            in_=ot[:, offs2[s]:offs2[s] + caps2[s]])

    nc.compile()
    return (nc, xb.name, x8.name, [w.name for w in wbsegs],
            [w.name for w in w8segs], out.name)


def kernel(X, G, W, b):
    global LAST_EXEC_NS
    from concourse.bass_utils import run_bass_kernel_spmd
    import ml_dtypes

    X = np.ascontiguousarray(np.asarray(X, dtype=np.float32))
    G = np.asarray(G, dtype=np.float32)
    W = np.asarray(W, dtype=np.float32)
    b = np.asarray(b, dtype=np.float32)
    n_tok, d_in = X.shape
    n_exp = G.shape[1]
    d_out = W.shape[1]

    # --- host gating: softmax over experts, top-2, renormalize ---
    g = G - G.max(axis=1, keepdims=True)
    sm = np.exp(g)
    sm /= sm.sum(axis=1, keepdims=True)
    top_idx = np.argsort(-sm, axis=1, kind="stable")[:, :TOP_K]  # ties -> lower index
    top_w = np.take_along_axis(sm, top_idx, axis=1)
    norm_w = top_w / top_w.sum(axis=1, keepdims=True)

    # --- dispatch by (expert, rank): rank-1 -> bf16 class, rank-2 -> fp8 ---
    cls_tokens = [[], []]
    cls_scales = [[], []]
    for e in range(n_exp):
        for k in range(TOP_K):
            m = top_idx[:, k] == e
            cl = 0 if k == 0 else 1
            cls_tokens[cl].append(np.where(m)[0])
            cls_scales[cl].append(norm_w[m, k].astype(np.float32))

    counts1 = [len(i) for i in cls_tokens[0]]
    counts2 = [len(i) for i in cls_tokens[1]]
    caps1, cores1 = _balance(counts1, n_cores=n_exp)
    caps2, cores2 = _balance(counts2, n_cores=n_exp)
    C1, C2 = sum(caps1), sum(caps2)
    offs1 = [sum(caps1[:i]) for i in range(len(caps1))]
    offs2 = [sum(caps2[:i]) for i in range(len(caps2))]

    _install_ntff_shim()  # harmless if unavailable; needed if tracing is on
    key = (caps1, caps2)
    if key not in _cache:
        _cache[key] = _build_v5(caps1, caps2)
    nc, xb_name, x8_name, wb_names, w8_names, out_name = _cache[key]

    # W tiled per expert, cached across cores:
    # w_t[ob, p, ks, o2] = W_e[ob*128+o2, ks*128+p]
    wb_tiled, w8_tiled = {}, {}

    def tile_w(e):
        return np.ascontiguousarray(
            W[e].reshape(OB, P, KSUB, P).transpose(0, 3, 2, 1))

    def get_wb(e):
        if e not in wb_tiled:
            wb_tiled[e] = tile_w(e).astype(ml_dtypes.bfloat16)
        return wb_tiled[e]

    def get_w8(e):
        if e not in w8_tiled:
            w8_tiled[e] = (tile_w(e) * W8_SCALE).astype(ml_dtypes.float8_e4m3)
        return w8_tiled[e]

    def build_x(pieces, offs, Csz, toks, scale):
        Xg = np.zeros((Csz, d_in), dtype=np.float32)
        for (e, off, ln), so in zip(pieces, offs):
            Xg[so:so + ln] = X[toks[e][off:off + ln]]
        if scale != 1.0:
            Xg *= scale
        return np.ascontiguousarray(Xg.reshape(Csz, KSUB, P).transpose(2, 1, 0))

    in_maps = []
    for ci in range(n_exp):
        xb_t = build_x(cores1[ci], offs1, C1, cls_tokens[0], 1.0).astype(
            ml_dtypes.bfloat16)
        x8_t = build_x(cores2[ci], offs2, C2, cls_tokens[1], X8_SCALE).astype(
            ml_dtypes.float8_e4m3)
        m = {xb_name: xb_t, x8_name: x8_t}
        for s in range(len(caps1)):
            m[wb_names[s]] = get_wb(cores1[ci][s][0])
        for s in range(len(caps2)):
            m[w8_names[s]] = get_w8(cores2[ci][s][0])
        in_maps.append(m)

    trace = bool(os.environ.get("KERNEL_TRACE"))
    res = run_bass_kernel_spmd(nc, in_maps, core_ids=list(range(n_exp)),
                               trace=trace)
    LAST_EXEC_NS = res.exec_time_ns

    # --- host combine: scatter-add with gate scale and bias ---
    inv8 = 1.0 / (X8_SCALE * W8_SCALE)
    Y = np.zeros((n_tok, d_out), dtype=np.float32)
    for ci in range(n_exp):
        arr = res.results[ci][out_name].reshape(d_out, C1 + C2).astype(
            np.float32)
        for cl, cores, offs, r_base, sc in (
            (0, cores1, offs1, 0, 1.0),
            (1, cores2, offs2, C1, inv8),
        ):
            for (e, off, ln), so in zip(cores[ci], offs):
                if ln == 0:
                    continue
                idx = cls_tokens[cl][e][off:off + ln]
                s = cls_scales[cl][e][off:off + ln][:, None]
                Ye = arr[:, r_base + so:r_base + so + ln].T
                if sc != 1.0:
                    Ye = Ye * sc
                Y[idx] += s * (Ye + b[e][None, :])
    return Y
